# revision 21
# baseline (speedup 1.0000x reference)
"""Causal self-attention (B=2, T=2048, D=1024, H=16, Dh=64) on 8 TRN2 cores.

Sharding: core c -> batch b = c//4 (data parallel), head group g = c%4
(tensor parallel, 4 heads = 256 dims). Each core computes a full-shape
[T, D] partial of the output projection for its (b, g); the host sums
the 4 head-group partials per batch (bf16 partials, fp32 host sum).

Compute dtype bf16 (host-cast inputs), fp32 PSUM accumulation.

v2: single software-pipelined stream instead of 3 sequential phases.
Attention region n (tq cols [512n, 512n+512)) only needs q/k/v from
token chunks <= n, so it starts right after proj chunk n. Proj chunk
n+1 and the output projection of region n-1 are woven between attention
steps as TensorE filler while ScalarE grinds the softmax exp (~78us
total, the non-TM bottleneck). PSUM: 4 banks sT double-buffer + 2 banks
oT + 2 banks shared proj/outproj rotation.
"""

import numpy as np
from contextlib import ExitStack

import concourse.bass as bass
import concourse.tile as tile
from concourse import bacc, mybir
from concourse.bass_utils import run_bass_kernel_spmd

F32 = mybir.dt.float32
BF16 = mybir.dt.bfloat16
CDT = BF16

B, T, D = 2, 2048, 1024
H_TOT, DH = 16, 64
HL = 4                # local heads per core
DG = HL * DH          # 256 local head dims
NT = T // 128         # 16 t-tiles
NCH = T // 512        # 4 t-chunks / tq regions
CT = D // 128         # 8 c-tiles

_CACHE = {}


def build():
    nc = bacc.Bacc("TRN2", target_bir_lowering=False, debug=False, num_devices=8)
    xT_d = nc.dram_tensor("xT", [NCH, 128, CT, 512], CDT, kind="ExternalInput").ap()
    wq_d = nc.dram_tensor("wq", [128, CT, DG], CDT, kind="ExternalInput").ap()
    wk_d = nc.dram_tensor("wk", [128, CT, DG], CDT, kind="ExternalInput").ap()
    wv_d = nc.dram_tensor("wv", [128, CT, DG], CDT, kind="ExternalInput").ap()
    wo_d = nc.dram_tensor("wo", [128, 2, D], CDT, kind="ExternalInput").ap()
    mask_d = nc.dram_tensor("mask", [128, 128], CDT, kind="ExternalInput").ap()
    out_d = nc.dram_tensor("out", [T, D], CDT, kind="ExternalOutput").ap()

    with tile.TileContext(nc) as tc:
        with ExitStack() as ctx:
            cons = ctx.enter_context(tc.tile_pool(name="cons", bufs=1))
            xp = ctx.enter_context(tc.tile_pool(name="xp", bufs=3))
            cp = ctx.enter_context(tc.tile_pool(name="cp", bufs=2))
            pp = ctx.enter_context(tc.tile_pool(name="pp", bufs=4))
            outp = ctx.enter_context(tc.tile_pool(name="outp", bufs=4))
            psS = ctx.enter_context(tc.tile_pool(name="psS", bufs=2, space="PSUM"))
            psO = ctx.enter_context(tc.tile_pool(name="psO", bufs=1, space="PSUM"))
            pgen = ctx.enter_context(tc.tile_pool(name="pgen", bufs=2, space="PSUM"))

            wq_sb = cons.tile([128, CT, DG], CDT)
            wk_sb = cons.tile([128, CT, DG], CDT)
            wv_sb = cons.tile([128, CT, DG], CDT)
            wo_sb = cons.tile([128, 2, D], CDT)
            mask_sb = cons.tile([128, 128], CDT)

            qsb = cons.tile([128, 2, T], CDT)
            ksb = cons.tile([128, 2, T], CDT)
            lrows = cons.tile([128, T], F32)
            v_sb = cons.tile([128, NT, HL, DH + 1], CDT)
            y_sb = cons.tile([128, 2, T], CDT)

            x_tiles = {}

            def dma_x0():
                x_sb = xp.tile([128, CT, 512], CDT, tag="x", name="x0")
                x_tiles[0] = x_sb
                nc.scalar.dma_start(x_sb[:, 0:4, :], xT_d[0, :, 0:4, :])
                nc.scalar.dma_start(x_sb[:, 4:8, :], xT_d[0, :, 4:8, :])

            def dma_w_qk():
                nc.sync.dma_start(wq_sb[:], wq_d[:])
                nc.gpsimd.dma_start(wk_sb[:], wk_d[:])

            def dma_x(n):
                x_sb = xp.tile([128, CT, 512], CDT, tag="x", name=f"x{n}")
                x_tiles[n] = x_sb
                nc.sync.dma_start(x_sb[:, 0:4, :], xT_d[n, :, 0:4, :])
                nc.gpsimd.dma_start(x_sb[:, 4:CT, :], xT_d[n, :, 4:CT, :])

            def proj_qk_group(n, w_sb, dst, di, j2):
                def f():
                    x_sb = x_tiles[n]
                    pq = pgen.tile(
                        [128, 512], F32, tag="gen", name=f"g_qk{n}_{di}_{j2}"
                    )
                    for ct in range(CT):
                        nc.tensor.matmul(
                            pq[:],
                            w_sb[:, ct, 128 * j2 : 128 * (j2 + 1)],
                            x_sb[:, ct, :],
                            start=(ct == 0),
                            stop=(ct == CT - 1),
                        )
                    nc.vector.tensor_copy(dst[:, j2, 512 * n : 512 * (n + 1)], pq[:])
                return f

            def proj_v_group(n, i):
                def f():
                    x_sb = x_tiles[n]
                    ti = 4 * n + i
                    pv = pgen.tile([128, 512], F32, tag="gen", name=f"g_v{ti}")
                    for ct in range(CT):
                        nc.tensor.matmul(
                            pv[:, 0:DG],
                            x_sb[:, ct, 128 * i : 128 * (i + 1)],
                            wv_sb[:, ct, :],
                            start=(ct == 0),
                            stop=(ct == CT - 1),
                        )
                    nc.vector.tensor_copy(
                        v_sb[:, ti, :, 0:DH],
                        pv[:, 0:DG].rearrange("p (h d) -> p h d", h=HL),
                    )
                return f

            def proj_groups(n):
                gs = []
                for di, (w_sb, dst) in enumerate(((wq_sb, qsb), (wk_sb, ksb))):
                    for j2 in range(2):
                        gs.append(proj_qk_group(n, w_sb, dst, di, j2))
                for i in range(4):
                    gs.append(proj_v_group(n, i))
                return gs

            def p3_group(reg, i, oc):
                def f():
                    ti = 4 * reg + i
                    po = pgen.tile([128, 512], F32, tag="gen", name=f"g_po{ti}_{oc}")
                    for g2 in range(2):
                        nc.tensor.matmul(
                            po[:],
                            y_sb[:, g2, 128 * ti : 128 * (ti + 1)],
                            wo_sb[:, g2, 512 * oc : 512 * (oc + 1)],
                            start=(g2 == 0),
                            stop=(g2 == 1),
                        )
                    o_sb = outp.tile([128, 512], CDT, tag="o", name=f"o{ti}_{oc}")
                    if reg == 3:
                        # tail: alternate drain engines (scalar is free) and
                        # split the store across queues
                        if (ti + oc) % 2 == 0:
                            nc.vector.tensor_copy(o_sb[:], po[:])
                        else:
                            nc.scalar.copy(o_sb[:], po[:])
                        engs = (nc.gpsimd, nc.sync, nc.scalar)
                        e0 = engs[(2 * ti + oc) % 3]
                        e1 = engs[(2 * ti + oc + 1) % 3]
                        e0.dma_start(
                            out_d[128 * ti : 128 * ti + 64, 512 * oc : 512 * (oc + 1)],
                            o_sb[0:64, :],
                        )
                        e1.dma_start(
                            out_d[128 * ti + 64 : 128 * (ti + 1), 512 * oc : 512 * (oc + 1)],
                            o_sb[64:128, :],
                        )
                    else:
                        nc.vector.tensor_copy(o_sb[:], po[:])
                        eng = nc.gpsimd if (ti + oc) % 2 == 0 else nc.sync
                        eng.dma_start(
                            out_d[128 * ti : 128 * (ti + 1), 512 * oc : 512 * (oc + 1)],
                            o_sb[:],
                        )
                return f

            def p3_groups(reg):
                return [p3_group(reg, i, oc) for i in range(4) for oc in range(2)]

            # region-3 out-proj split by pair: pair-0 halves run as woven
            # fillers once p0's norm lands (mid-region); after the final norm
            # only pair-1 matmul + add + store remain per tile
            o0_sb = cons.tile([128, 8, 512], F32)

            def p3a_group(i, oc):
                def f():
                    ti = 12 + i
                    po = pgen.tile([128, 512], F32, tag="gen", name=f"g_pa{ti}_{oc}")
                    nc.tensor.matmul(
                        po[:],
                        y_sb[:, 0, 128 * ti : 128 * (ti + 1)],
                        wo_sb[:, 0, 512 * oc : 512 * (oc + 1)],
                        start=True,
                        stop=True,
                    )
                    nc.vector.tensor_copy(o0_sb[:, 2 * i + oc, :], po[:])
                return f

            def p3b_group(i, oc):
                def f():
                    ti = 12 + i
                    po = pgen.tile([128, 512], F32, tag="gen", name=f"g_pb{ti}_{oc}")
                    nc.tensor.matmul(
                        po[:],
                        y_sb[:, 1, 128 * ti : 128 * (ti + 1)],
                        wo_sb[:, 1, 512 * oc : 512 * (oc + 1)],
                        start=True,
                        stop=True,
                    )
                    o_sb = outp.tile([128, 512], CDT, tag="o", name=f"o{ti}_{oc}")
                    nc.vector.tensor_add(o_sb[:], po[:], o0_sb[:, 2 * i + oc, :])
                    engs = (nc.gpsimd, nc.sync, nc.scalar)
                    e0 = engs[(2 * i + oc) % 3]
                    e1 = engs[(2 * i + oc + 1) % 3]
                    e0.dma_start(
                        out_d[128 * ti : 128 * ti + 64, 512 * oc : 512 * (oc + 1)],
                        o_sb[0:64, :],
                    )
                    e1.dma_start(
                        out_d[128 * ti + 64 : 128 * (ti + 1), 512 * oc : 512 * (oc + 1)],
                        o_sb[64:128, :],
                    )
                return f

            # ---- attention region (both pairs) with woven filler work ----
            def attention_region(reg, fillers, front=()):
                c0r, c1r = 512 * reg, 512 * (reg + 1)
                jlast = 4 * reg + 3
                steps_total = 2 * (jlast + 2)
                fq = list(front) + list(fillers)
                nfront = len(front)
                state = {"fi": 0, "step": 0}

                def tick():
                    state["step"] += 1
                    want = max(
                        min(state["step"], nfront),
                        (len(fq) * state["step"]) // steps_total,
                    )
                    while state["fi"] < want:
                        fq[state["fi"]]()
                        state["fi"] += 1

                for p in range(2):
                    oTa = psO.tile([DH + 1, 512], F32, tag="oTa", name=f"oTa_{p}_{reg}")
                    oTb = psO.tile([DH + 1, 512], F32, tag="oTb", name=f"oTb_{p}_{reg}")

                    def emit_st(j):
                        c0 = max(c0r, 128 * j)
                        w = c1r - c0
                        sT = psS.tile(
                            [128, 1024], F32, tag="sT", name=f"sT{p}_{reg}_{j}"
                        )
                        nc.tensor.matmul(
                            sT[:, 0:w],
                            ksb[0:DH, p, 128 * j : 128 * (j + 1)],
                            qsb[0:DH, p, c0:c1r],
                            start=True,
                            stop=True,
                        )
                        nc.tensor.matmul(
                            sT[:, 512 : 512 + w],
                            ksb[DH:128, p, 128 * j : 128 * (j + 1)],
                            qsb[DH:128, p, c0:c1r],
                            start=True,
                            stop=True,
                        )
                        pT = pp.tile(
                            [128, 1024], CDT, tag="pT", name=f"pT{p}_{reg}_{j}"
                        )
                        nc.scalar.activation(
                            pT[:, 0 : 512 + w],
                            sT[:, 0 : 512 + w],
                            mybir.ActivationFunctionType.Exp,
                            scale=0.125,
                        )
                        if j >= 4 * reg:  # diagonal block at rel cols [0,128)
                            nc.vector.tensor_mul(
                                pT[:, 0:128], pT[:, 0:128], mask_sb[:]
                            )
                            nc.vector.tensor_mul(
                                pT[:, 512:640], pT[:, 512:640], mask_sb[:]
                            )
                        return pT

                    def emit_pv(j, pT):
                        c0 = max(c0r, 128 * j)
                        w = c1r - c0
                        nc.tensor.matmul(
                            oTa[:, c0 - c0r :],
                            v_sb[:, j, 2 * p, :],
                            pT[:, 0:w],
                            start=(j == 0),
                            stop=(j == jlast),
                            skip_group_check=True,
                        )
                        nc.tensor.matmul(
                            oTb[:, c0 - c0r :],
                            v_sb[:, j, 2 * p + 1, :],
                            pT[:, 512 : 512 + w],
                            start=(j == 0),
                            stop=(j == jlast),
                            skip_group_check=True,
                        )

                    def emit_norm():
                        # softmax denominator: l rows out first (frees oT banks
                        # and starts the reshape-dma chain early), one shared
                        # reciprocal, per-head broadcast + multiply. The last
                        # region's pair-1 chain is the exposed tail: second
                        # head's copy goes on the idle scalar engine, dmas on
                        # two queues, broadcast/mul in 256-col pieces.
                        fast = reg == 3 and p == 1
                        engs = (nc.sync, nc.scalar) if fast else (nc.sync, nc.sync)
                        for idx, oT in ((0, oTa), (1, oTb)):
                            h = 2 * p + idx
                            if fast and idx == 1:
                                nc.scalar.copy(
                                    lrows[32 * h : 32 * h + 1, c0r:c1r],
                                    oT[DH : DH + 1, :],
                                )
                            else:
                                nc.vector.tensor_copy(
                                    lrows[32 * h : 32 * h + 1, c0r:c1r],
                                    oT[DH : DH + 1, :],
                                )
                            engs[idx].dma_start(
                                lt_sb[32 * reg : 32 * (reg + 1), 16 * idx : 16 * idx + 16],
                                lrows[32 * h : 32 * h + 1, c0r:c1r],
                            )
                        for idx, oT in ((0, oTa), (1, oTb)):
                            hp = 64 * idx
                            nc.vector.tensor_copy(
                                y_sb[hp : hp + DH, p, c0r:c1r], oT[0:DH, :]
                            )
                        if fast:
                            for idx, r1 in ((0, ra_sb), (1, rc_sb)):
                                nc.vector.reciprocal(
                                    rt_sb[32 * reg : 32 * (reg + 1), 16 * idx : 16 * idx + 16],
                                    lt_sb[32 * reg : 32 * (reg + 1), 16 * idx : 16 * idx + 16],
                                )
                                engs[idx].dma_start(
                                    r1[:, c0r:c1r],
                                    rt_sb[32 * reg : 32 * (reg + 1), 16 * idx : 16 * idx + 16],
                                )
                        else:
                            nc.vector.reciprocal(
                                rt_sb[32 * reg : 32 * (reg + 1), :],
                                lt_sb[32 * reg : 32 * (reg + 1), :],
                            )
                            for idx, r1 in ((0, ra_sb), (1, rc_sb)):
                                engs[idx].dma_start(
                                    r1[:, c0r:c1r],
                                    rt_sb[32 * reg : 32 * (reg + 1), 16 * idx : 16 * idx + 16],
                                )
                        pieces = (
                            ((c0r, c0r + 256), (c0r + 256, c1r)) if fast else ((c0r, c1r),)
                        )
                        for pc0, pc1 in pieces:
                            for idx, (r1, rb1) in (
                                (0, (ra_sb, rbb_sb)),
                                (1, (rc_sb, rcc_sb)),
                            ):
                                hp = 64 * idx
                                nc.gpsimd.partition_broadcast(
                                    rb1[0 : hp + DH, pc0:pc1], r1[:, pc0:pc1]
                                )
                                nc.vector.tensor_mul(
                                    y_sb[hp : hp + DH, p, pc0:pc1],
                                    y_sb[hp : hp + DH, p, pc0:pc1],
                                    rb1[hp : hp + DH, pc0:pc1],
                                )

                    lt_sb = cp.tile([128, 32], F32, tag="lt", name=f"lt{p}_{reg}")
                    rt_sb = cp.tile([128, 32], F32, tag="rt", name=f"rt{p}_{reg}")
                    ra_sb = cp.tile([1, T], F32, tag="ra", name=f"ra{p}_{reg}")
                    rc_sb = cp.tile([1, T], F32, tag="rc", name=f"rc{p}_{reg}")
                    rbb_sb = cp.tile([128, T], F32, tag="rb", name=f"rb{p}_{reg}")
                    rcc_sb = cp.tile([128, T], F32, tag="rc2", name=f"rc2{p}_{reg}")

                    prev = None
                    for j in range(jlast + 1):
                        pT = emit_st(j)
                        if prev is not None:
                            emit_pv(*prev)
                        prev = (j, pT)
                        tick()
                    emit_pv(*prev)
                    emit_norm()
                    tick()

            # ---- schedule ----
            # lead-in: only wq/wk/x0 ahead of first matmuls; rest behind
            dma_w_qk()
            dma_x0()
            nc.vector.memset(v_sb[:, :, :, DH], 1.0)
            g0 = proj_groups(0)
            for g in g0[0:4]:   # chunk-0 QK
                g()
            nc.sync.dma_start(wv_sb[:], wv_d[:])
            nc.sync.dma_start(mask_sb[:], mask_d[:])
            for g in g0[4:8]:   # chunk-0 V
                g()
            nc.gpsimd.dma_start(wo_sb[:], wo_d[:])
            dma_x(1)
            dma_x(2)
            attention_region(0, proj_groups(1))
            dma_x(3)
            attention_region(1, proj_groups(2) + p3_groups(0))
            g3 = proj_groups(3)
            p32 = p3_groups(2)
            attention_region(2, g3[0:4] + p3_groups(1))   # chunk-3 QK
            # chunk-3 V front-loaded: must precede the PV j>=12 consumers
            attention_region(3, p32, front=g3[4:8])
            for i in range(4):   # pair-0 halves: no dependency on the final
                for oc in range(2):  # norm -> they fill the norm-chain wait
                    p3a_group(i, oc)()
            for i in range(4):
                for oc in range(2):
                    p3b_group(i, oc)()
    nc.compile()
    return nc


def make_in_maps(x, Wq, Wk, Wv, Wo):
    import ml_dtypes

    cnp = ml_dtypes.bfloat16
    mask = np.triu(np.ones((128, 128), dtype=cnp))  # [tk, tq] valid tk<=tq
    in_maps = []
    for c in range(8):
        b, g = c // 4, c % 4
        rows = slice(DG * g, DG * (g + 1))
        in_maps.append(
            {
                "xT": np.ascontiguousarray(
                    x[b].T.reshape(CT, 128, NCH, 512).transpose(2, 1, 0, 3)
                ).astype(cnp),
                "wq": np.ascontiguousarray(
                    Wq[rows].T.reshape(CT, 128, DG).transpose(1, 0, 2)
                ).astype(cnp),
                "wk": np.ascontiguousarray(
                    Wk[rows].T.reshape(CT, 128, DG).transpose(1, 0, 2)
                ).astype(cnp),
                "wv": np.ascontiguousarray(
                    Wv[rows].T.reshape(CT, 128, DG).transpose(1, 0, 2)
                ).astype(cnp),
                "wo": np.ascontiguousarray(
                    Wo[:, rows].T.reshape(2, 128, D).transpose(1, 0, 2)
                ).astype(cnp),
                "mask": mask,
            }
        )
    return in_maps


def _run(x, Wq, Wk, Wv, Wo, trace=False):
    if "nc" not in _CACHE:
        _CACHE["nc"] = build()
    nc = _CACHE["nc"]
    in_maps = make_in_maps(x, Wq, Wk, Wv, Wo)
    res = run_bass_kernel_spmd(nc, in_maps, core_ids=list(range(8)), trace=trace)
    out = np.zeros((B, T, D), dtype=np.float32)
    for c in range(8):
        out[c // 4] += res.results[c]["out"].astype(np.float32)
    return out, res


def kernel(x, Wq, Wk, Wv, Wo):
    out, _ = _run(
        np.asarray(x, dtype=np.float32),
        np.asarray(Wq, dtype=np.float32),
        np.asarray(Wk, dtype=np.float32),
        np.asarray(Wv, dtype=np.float32),
        np.asarray(Wo, dtype=np.float32),
    )
    return out


# revision 22
# speedup vs baseline: 1.0232x; 1.0232x over previous
"""Causal self-attention (B=2, T=2048, D=1024, H=16, Dh=64) on 8 TRN2 cores.

Sharding: core c -> batch b = c//4 (data parallel), head group g = c%4
(tensor parallel, 4 heads = 256 dims). Each core computes a full-shape
[T, D] partial of the output projection for its (b, g); the host sums
the 4 head-group partials per batch (bf16 partials, fp32 host sum).

Compute dtype bf16 (host-cast inputs), fp32 PSUM accumulation.

v2: single software-pipelined stream instead of 3 sequential phases.
Attention region n (tq cols [512n, 512n+512)) only needs q/k/v from
token chunks <= n, so it starts right after proj chunk n. Proj chunk
n+1 and the output projection of region n-1 are woven between attention
steps as TensorE filler while ScalarE grinds the softmax exp (~78us
total, the non-TM bottleneck). PSUM: 4 banks sT double-buffer + 2 banks
oT + 2 banks shared proj/outproj rotation.
"""

import numpy as np
from contextlib import ExitStack

import concourse.bass as bass
import concourse.tile as tile
from concourse import bacc, mybir
from concourse.bass_utils import run_bass_kernel_spmd

F32 = mybir.dt.float32
BF16 = mybir.dt.bfloat16
CDT = BF16

B, T, D = 2, 2048, 1024
H_TOT, DH = 16, 64
HL = 4                # local heads per core
DG = HL * DH          # 256 local head dims
NT = T // 128         # 16 t-tiles
NCH = T // 512        # 4 t-chunks / tq regions
CT = D // 128         # 8 c-tiles

_CACHE = {}


def build():
    nc = bacc.Bacc("TRN2", target_bir_lowering=False, debug=False, num_devices=8)
    xT_d = nc.dram_tensor("xT", [NCH, 128, CT, 512], CDT, kind="ExternalInput").ap()
    wq_d = nc.dram_tensor("wq", [128, CT, DG], CDT, kind="ExternalInput").ap()
    wk_d = nc.dram_tensor("wk", [128, CT, DG], CDT, kind="ExternalInput").ap()
    wv_d = nc.dram_tensor("wv", [128, CT, DG], CDT, kind="ExternalInput").ap()
    wo_d = nc.dram_tensor("wo", [128, 2, D], CDT, kind="ExternalInput").ap()
    mask_d = nc.dram_tensor("mask", [128, 128], CDT, kind="ExternalInput").ap()
    out_d = nc.dram_tensor("out", [T, D], CDT, kind="ExternalOutput").ap()

    with tile.TileContext(nc) as tc:
        with ExitStack() as ctx:
            cons = ctx.enter_context(tc.tile_pool(name="cons", bufs=1))
            xp = ctx.enter_context(tc.tile_pool(name="xp", bufs=3))
            cp = ctx.enter_context(tc.tile_pool(name="cp", bufs=2))
            pp = ctx.enter_context(tc.tile_pool(name="pp", bufs=4))
            outp = ctx.enter_context(tc.tile_pool(name="outp", bufs=4))
            psS = ctx.enter_context(tc.tile_pool(name="psS", bufs=2, space="PSUM"))
            psO = ctx.enter_context(tc.tile_pool(name="psO", bufs=1, space="PSUM"))
            pgen = ctx.enter_context(tc.tile_pool(name="pgen", bufs=2, space="PSUM"))

            wq_sb = cons.tile([128, CT, DG], CDT)
            wk_sb = cons.tile([128, CT, DG], CDT)
            wv_sb = cons.tile([128, CT, DG], CDT)
            wo_sb = cons.tile([128, 2, D], CDT)
            mask_sb = cons.tile([128, 128], CDT)

            qsb = cons.tile([128, 2, T], CDT)
            ksb = cons.tile([128, 2, T], CDT)
            lrows = cons.tile([128, T], F32)
            v_sb = cons.tile([128, NT, HL, DH + 1], CDT)
            y_sb = cons.tile([128, 2, T], CDT)

            x_tiles = {}

            def dma_x0():
                x_sb = xp.tile([128, CT, 512], CDT, tag="x", name="x0")
                x_tiles[0] = x_sb
                nc.scalar.dma_start(x_sb[:, 0:4, :], xT_d[0, :, 0:4, :])
                nc.scalar.dma_start(x_sb[:, 4:8, :], xT_d[0, :, 4:8, :])

            def dma_w_qk():
                nc.sync.dma_start(wq_sb[:], wq_d[:])
                nc.gpsimd.dma_start(wk_sb[:], wk_d[:])

            def dma_x(n):
                x_sb = xp.tile([128, CT, 512], CDT, tag="x", name=f"x{n}")
                x_tiles[n] = x_sb
                nc.sync.dma_start(x_sb[:, 0:4, :], xT_d[n, :, 0:4, :])
                nc.gpsimd.dma_start(x_sb[:, 4:CT, :], xT_d[n, :, 4:CT, :])

            def proj_qk_group(n, w_sb, dst, di, j2):
                def f():
                    x_sb = x_tiles[n]
                    pq = pgen.tile(
                        [128, 512], F32, tag="gen", name=f"g_qk{n}_{di}_{j2}"
                    )
                    for ct in range(CT):
                        nc.tensor.matmul(
                            pq[:],
                            w_sb[:, ct, 128 * j2 : 128 * (j2 + 1)],
                            x_sb[:, ct, :],
                            start=(ct == 0),
                            stop=(ct == CT - 1),
                        )
                    nc.vector.tensor_copy(dst[:, j2, 512 * n : 512 * (n + 1)], pq[:])
                return f

            def proj_v_group(n, i):
                def f():
                    x_sb = x_tiles[n]
                    ti = 4 * n + i
                    pv = pgen.tile([128, 512], F32, tag="gen", name=f"g_v{ti}")
                    for ct in range(CT):
                        nc.tensor.matmul(
                            pv[:, 0:DG],
                            x_sb[:, ct, 128 * i : 128 * (i + 1)],
                            wv_sb[:, ct, :],
                            start=(ct == 0),
                            stop=(ct == CT - 1),
                        )
                    nc.vector.tensor_copy(
                        v_sb[:, ti, :, 0:DH],
                        pv[:, 0:DG].rearrange("p (h d) -> p h d", h=HL),
                    )
                return f

            def proj_groups(n):
                gs = []
                for di, (w_sb, dst) in enumerate(((wq_sb, qsb), (wk_sb, ksb))):
                    for j2 in range(2):
                        gs.append(proj_qk_group(n, w_sb, dst, di, j2))
                for i in range(4):
                    gs.append(proj_v_group(n, i))
                return gs

            def p3_group(reg, i, oc):
                def f():
                    ti = 4 * reg + i
                    po = pgen.tile([128, 512], F32, tag="gen", name=f"g_po{ti}_{oc}")
                    for g2 in range(2):
                        nc.tensor.matmul(
                            po[:],
                            y_sb[:, g2, 128 * ti : 128 * (ti + 1)],
                            wo_sb[:, g2, 512 * oc : 512 * (oc + 1)],
                            start=(g2 == 0),
                            stop=(g2 == 1),
                        )
                    o_sb = outp.tile([128, 512], CDT, tag="o", name=f"o{ti}_{oc}")
                    if reg == 3:
                        # tail: alternate drain engines (scalar is free) and
                        # split the store across queues
                        if (ti + oc) % 2 == 0:
                            nc.vector.tensor_copy(o_sb[:], po[:])
                        else:
                            nc.scalar.copy(o_sb[:], po[:])
                        engs = (nc.gpsimd, nc.sync, nc.scalar)
                        e0 = engs[(2 * ti + oc) % 3]
                        e1 = engs[(2 * ti + oc + 1) % 3]
                        e0.dma_start(
                            out_d[128 * ti : 128 * ti + 64, 512 * oc : 512 * (oc + 1)],
                            o_sb[0:64, :],
                        )
                        e1.dma_start(
                            out_d[128 * ti + 64 : 128 * (ti + 1), 512 * oc : 512 * (oc + 1)],
                            o_sb[64:128, :],
                        )
                    else:
                        nc.vector.tensor_copy(o_sb[:], po[:])
                        eng = nc.gpsimd if (ti + oc) % 2 == 0 else nc.sync
                        eng.dma_start(
                            out_d[128 * ti : 128 * (ti + 1), 512 * oc : 512 * (oc + 1)],
                            o_sb[:],
                        )
                return f

            def p3_groups(reg):
                return [p3_group(reg, i, oc) for i in range(4) for oc in range(2)]

            # region-3 out-proj split by pair: pair-0 halves run as woven
            # fillers once p0's norm lands (mid-region); after the final norm
            # only pair-1 matmul + add + store remain per tile
            o0_sb = cons.tile([128, 8, 512], F32)

            def p3a_group(i, oc):
                def f():
                    ti = 12 + i
                    po = pgen.tile([128, 512], F32, tag="gen", name=f"g_pa{ti}_{oc}")
                    nc.tensor.matmul(
                        po[:],
                        y_sb[:, 0, 128 * ti : 128 * (ti + 1)],
                        wo_sb[:, 0, 512 * oc : 512 * (oc + 1)],
                        start=True,
                        stop=True,
                    )
                    nc.scalar.copy(o0_sb[:, 2 * i + oc, :], po[:])
                return f

            def p3b_group(i, oc):
                def f():
                    ti = 12 + i
                    po = pgen.tile([128, 512], F32, tag="gen", name=f"g_pb{ti}_{oc}")
                    nc.tensor.matmul(
                        po[:],
                        y_sb[:, 1, 128 * ti : 128 * (ti + 1)],
                        wo_sb[:, 1, 512 * oc : 512 * (oc + 1)],
                        start=True,
                        stop=True,
                    )
                    o_sb = outp.tile([128, 512], CDT, tag="o", name=f"o{ti}_{oc}")
                    nc.vector.tensor_add(o_sb[:], po[:], o0_sb[:, 2 * i + oc, :])
                    engs = (nc.gpsimd, nc.sync, nc.scalar)
                    e0 = engs[(2 * i + oc) % 3]
                    e1 = engs[(2 * i + oc + 1) % 3]
                    e0.dma_start(
                        out_d[128 * ti : 128 * ti + 64, 512 * oc : 512 * (oc + 1)],
                        o_sb[0:64, :],
                    )
                    e1.dma_start(
                        out_d[128 * ti + 64 : 128 * (ti + 1), 512 * oc : 512 * (oc + 1)],
                        o_sb[64:128, :],
                    )
                return f

            # ---- attention region (both pairs) with woven filler work ----
            def attention_region(reg, fillers, front=()):
                c0r, c1r = 512 * reg, 512 * (reg + 1)
                jlast = 4 * reg + 3
                steps_total = 2 * (jlast + 2)
                fq = list(front) + list(fillers)
                nfront = len(front)
                state = {"fi": 0, "step": 0}

                def tick():
                    state["step"] += 1
                    want = max(
                        min(state["step"], nfront),
                        (len(fq) * state["step"]) // steps_total,
                    )
                    while state["fi"] < want:
                        fq[state["fi"]]()
                        state["fi"] += 1

                for p in range(2):
                    oTa = psO.tile([DH + 1, 512], F32, tag="oTa", name=f"oTa_{p}_{reg}")
                    oTb = psO.tile([DH + 1, 512], F32, tag="oTb", name=f"oTb_{p}_{reg}")

                    def emit_st(j):
                        c0 = max(c0r, 128 * j)
                        w = c1r - c0
                        sT = psS.tile(
                            [128, 1024], F32, tag="sT", name=f"sT{p}_{reg}_{j}"
                        )
                        nc.tensor.matmul(
                            sT[:, 0:w],
                            ksb[0:DH, p, 128 * j : 128 * (j + 1)],
                            qsb[0:DH, p, c0:c1r],
                            start=True,
                            stop=True,
                        )
                        nc.tensor.matmul(
                            sT[:, 512 : 512 + w],
                            ksb[DH:128, p, 128 * j : 128 * (j + 1)],
                            qsb[DH:128, p, c0:c1r],
                            start=True,
                            stop=True,
                        )
                        pT = pp.tile(
                            [128, 1024], CDT, tag="pT", name=f"pT{p}_{reg}_{j}"
                        )
                        nc.scalar.activation(
                            pT[:, 0 : 512 + w],
                            sT[:, 0 : 512 + w],
                            mybir.ActivationFunctionType.Exp,
                            scale=0.125,
                        )
                        if j >= 4 * reg:  # diagonal block at rel cols [0,128)
                            nc.vector.tensor_mul(
                                pT[:, 0:128], pT[:, 0:128], mask_sb[:]
                            )
                            nc.vector.tensor_mul(
                                pT[:, 512:640], pT[:, 512:640], mask_sb[:]
                            )
                        return pT

                    def emit_pv(j, pT):
                        c0 = max(c0r, 128 * j)
                        w = c1r - c0
                        nc.tensor.matmul(
                            oTa[:, c0 - c0r :],
                            v_sb[:, j, 2 * p, :],
                            pT[:, 0:w],
                            start=(j == 0),
                            stop=(j == jlast),
                            skip_group_check=True,
                        )
                        nc.tensor.matmul(
                            oTb[:, c0 - c0r :],
                            v_sb[:, j, 2 * p + 1, :],
                            pT[:, 512 : 512 + w],
                            start=(j == 0),
                            stop=(j == jlast),
                            skip_group_check=True,
                        )

                    def emit_norm():
                        # softmax denominator: l rows out first (frees oT banks
                        # and starts the reshape-dma chain early), one shared
                        # reciprocal, per-head broadcast + multiply. The last
                        # region's pair-1 chain is the exposed tail: second
                        # head's copy goes on the idle scalar engine, dmas on
                        # two queues, broadcast/mul in 256-col pieces.
                        fast = reg == 3 and p == 1
                        engs = (nc.sync, nc.scalar) if fast else (nc.sync, nc.sync)
                        for idx, oT in ((0, oTa), (1, oTb)):
                            h = 2 * p + idx
                            if fast and idx == 1:
                                nc.scalar.copy(
                                    lrows[32 * h : 32 * h + 1, c0r:c1r],
                                    oT[DH : DH + 1, :],
                                )
                            else:
                                nc.vector.tensor_copy(
                                    lrows[32 * h : 32 * h + 1, c0r:c1r],
                                    oT[DH : DH + 1, :],
                                )
                            engs[idx].dma_start(
                                lt_sb[32 * reg : 32 * (reg + 1), 16 * idx : 16 * idx + 16],
                                lrows[32 * h : 32 * h + 1, c0r:c1r],
                            )
                        for idx, oT in ((0, oTa), (1, oTb)):
                            hp = 64 * idx
                            nc.vector.tensor_copy(
                                y_sb[hp : hp + DH, p, c0r:c1r], oT[0:DH, :]
                            )
                        if fast:
                            for idx, r1 in ((0, ra_sb), (1, rc_sb)):
                                nc.vector.reciprocal(
                                    rt_sb[32 * reg : 32 * (reg + 1), 16 * idx : 16 * idx + 16],
                                    lt_sb[32 * reg : 32 * (reg + 1), 16 * idx : 16 * idx + 16],
                                )
                                engs[idx].dma_start(
                                    r1[:, c0r:c1r],
                                    rt_sb[32 * reg : 32 * (reg + 1), 16 * idx : 16 * idx + 16],
                                )
                        else:
                            nc.vector.reciprocal(
                                rt_sb[32 * reg : 32 * (reg + 1), :],
                                lt_sb[32 * reg : 32 * (reg + 1), :],
                            )
                            for idx, r1 in ((0, ra_sb), (1, rc_sb)):
                                engs[idx].dma_start(
                                    r1[:, c0r:c1r],
                                    rt_sb[32 * reg : 32 * (reg + 1), 16 * idx : 16 * idx + 16],
                                )
                        pieces = (
                            ((c0r, c0r + 256), (c0r + 256, c1r)) if fast else ((c0r, c1r),)
                        )
                        for pc0, pc1 in pieces:
                            for idx, (r1, rb1) in (
                                (0, (ra_sb, rbb_sb)),
                                (1, (rc_sb, rcc_sb)),
                            ):
                                hp = 64 * idx
                                nc.gpsimd.partition_broadcast(
                                    rb1[0 : hp + DH, pc0:pc1], r1[:, pc0:pc1]
                                )
                                nc.vector.tensor_mul(
                                    y_sb[hp : hp + DH, p, pc0:pc1],
                                    y_sb[hp : hp + DH, p, pc0:pc1],
                                    rb1[hp : hp + DH, pc0:pc1],
                                )

                    lt_sb = cp.tile([128, 32], F32, tag="lt", name=f"lt{p}_{reg}")
                    rt_sb = cp.tile([128, 32], F32, tag="rt", name=f"rt{p}_{reg}")
                    ra_sb = cp.tile([1, T], F32, tag="ra", name=f"ra{p}_{reg}")
                    rc_sb = cp.tile([1, T], F32, tag="rc", name=f"rc{p}_{reg}")
                    rbb_sb = cp.tile([128, T], F32, tag="rb", name=f"rb{p}_{reg}")
                    rcc_sb = cp.tile([128, T], F32, tag="rc2", name=f"rc2{p}_{reg}")

                    prev = None
                    for j in range(jlast + 1):
                        pT = emit_st(j)
                        if prev is not None:
                            emit_pv(*prev)
                        prev = (j, pT)
                        tick()
                    emit_pv(*prev)
                    tick()
                    emit_norm()

            # ---- schedule ----
            # lead-in: only wq/wk/x0 ahead of first matmuls; rest behind
            dma_w_qk()
            dma_x0()
            nc.vector.memset(v_sb[:, :, :, DH], 1.0)
            g0 = proj_groups(0)
            for g in g0[0:4]:   # chunk-0 QK
                g()
            nc.sync.dma_start(wv_sb[:], wv_d[:])
            nc.sync.dma_start(mask_sb[:], mask_d[:])
            for g in g0[4:8]:   # chunk-0 V
                g()
            nc.gpsimd.dma_start(wo_sb[:], wo_d[:])
            dma_x(1)
            dma_x(2)
            attention_region(0, proj_groups(1))
            dma_x(3)
            attention_region(1, proj_groups(2) + p3_groups(0))
            g3 = proj_groups(3)
            p32 = p3_groups(2)
            attention_region(2, g3[0:4] + p3_groups(1))   # chunk-3 QK
            # chunk-3 V front-loaded: must precede the PV j>=12 consumers
            attention_region(3, p32, front=g3[4:8])
            for i in range(4):   # pair-0 halves: no dependency on the final
                for oc in range(2):  # norm -> they fill the norm-chain wait
                    p3a_group(i, oc)()
            for i in range(4):
                for oc in range(2):
                    p3b_group(i, oc)()
    nc.compile()
    return nc


def make_in_maps(x, Wq, Wk, Wv, Wo):
    import ml_dtypes

    cnp = ml_dtypes.bfloat16
    mask = np.triu(np.ones((128, 128), dtype=cnp))  # [tk, tq] valid tk<=tq
    in_maps = []
    for c in range(8):
        b, g = c // 4, c % 4
        rows = slice(DG * g, DG * (g + 1))
        in_maps.append(
            {
                "xT": np.ascontiguousarray(
                    x[b].T.reshape(CT, 128, NCH, 512).transpose(2, 1, 0, 3)
                ).astype(cnp),
                "wq": np.ascontiguousarray(
                    Wq[rows].T.reshape(CT, 128, DG).transpose(1, 0, 2)
                ).astype(cnp),
                "wk": np.ascontiguousarray(
                    Wk[rows].T.reshape(CT, 128, DG).transpose(1, 0, 2)
                ).astype(cnp),
                "wv": np.ascontiguousarray(
                    Wv[rows].T.reshape(CT, 128, DG).transpose(1, 0, 2)
                ).astype(cnp),
                "wo": np.ascontiguousarray(
                    Wo[:, rows].T.reshape(2, 128, D).transpose(1, 0, 2)
                ).astype(cnp),
                "mask": mask,
            }
        )
    return in_maps


def _run(x, Wq, Wk, Wv, Wo, trace=False):
    if "nc" not in _CACHE:
        _CACHE["nc"] = build()
    nc = _CACHE["nc"]
    in_maps = make_in_maps(x, Wq, Wk, Wv, Wo)
    res = run_bass_kernel_spmd(nc, in_maps, core_ids=list(range(8)), trace=trace)
    out = np.zeros((B, T, D), dtype=np.float32)
    for c in range(8):
        out[c // 4] += res.results[c]["out"].astype(np.float32)
    return out, res


def kernel(x, Wq, Wk, Wv, Wo):
    out, _ = _run(
        np.asarray(x, dtype=np.float32),
        np.asarray(Wq, dtype=np.float32),
        np.asarray(Wk, dtype=np.float32),
        np.asarray(Wv, dtype=np.float32),
        np.asarray(Wo, dtype=np.float32),
    )
    return out
